# revision 9
# baseline (speedup 1.0000x reference)
"""Causal single-head attention block for Trainium2, SPMD across 8 NeuronCores.

Problem (hardcoded):
    x:     [4, 2048, 1024] f32
    w_qkv: [1024, 3072]    f32   (q | k | v column blocks)
    w_out: [1024, 1024]    f32
    b_out: [1024]          f32
    y = softmax(causal(q @ k.T / 32)) @ v @ w_out + b_out     -> [4, 2048, 1024]

Sharding: 2 cores per batch element. Within a batch, the 16 query subtiles of
128 rows are dealt round-robin to the core pair (core parity h gets subtiles
s = 2k + h, k = 0..7) so both cores see the identical causal work profile
(key-chunk counts [1,1,2,2,3,3,4,4]) and a single SPMD program serves all 8
cores; per-core behavior differs only through input data.

K/V production is split across the pair: each core projects only its parity's
two 512-row chunks (global chunks h and 2+h) and the pair exchanges them with
two AllGathers (one per 1024-row half, so the first half arrives while the
second is still being produced). K^T lives in SBUF; V is streamed from DRAM
during attention. All matmuls run in float32r (TF32-like PE mode, 4x the
fp32 matmul rate).
"""

import numpy as np

import concourse.mybir as mybir
import concourse.tile as tile
from concourse import bacc
from concourse.bass_utils import run_bass_kernel_spmd

FP32 = mybir.dt.float32
FP32R = mybir.dt.float32r
AF = mybir.ActivationFunctionType
ALU = mybir.AluOpType

B, S, D, NI, NO = 4, 2048, 1024, 1024, 1024
NCORES = 8
P = 128
DC = D // P    # 8 contraction chunks for the projections
IC = NI // P   # 8 inner-dim chunks
NSUB = 8       # local 128-row query subtiles per core
CC = [k // 2 + 1 for k in range(NSUB)]  # 512-key chunks per local subtile
SCALE = float(NI) ** -0.5
NEG = -1.0e9
PAIRS = [[2 * b, 2 * b + 1] for b in range(B)]

_CACHED = {}


def _build():
    nc = bacc.Bacc(None, target_bir_lowering=False, debug=False, num_devices=NCORES)

    xM = nc.dram_tensor("xM", [D, 1024], FP32R, kind="ExternalInput").ap()
    xQ = nc.dram_tensor("xQ", [D, NSUB * P], FP32R, kind="ExternalInput").ap()
    wk_d = nc.dram_tensor("wk", [D, NI], FP32R, kind="ExternalInput").ap()
    wv_d = nc.dram_tensor("wv", [D, NI], FP32R, kind="ExternalInput").ap()
    wq_d = nc.dram_tensor("wq", [D, NI], FP32R, kind="ExternalInput").ap()
    wo_d = nc.dram_tensor("wo", [NI, NO], FP32R, kind="ExternalInput").ap()
    masks = nc.dram_tensor("masks", [NSUB, P, 512], FP32, kind="ExternalInput").ap()
    bb = nc.dram_tensor("bb", [P, NO], FP32, kind="ExternalInput").ap()
    ident = nc.dram_tensor("ident", [P, P], FP32R, kind="ExternalInput").ap()
    y = nc.dram_tensor("y", [NSUB * P, NO], FP32, kind="ExternalOutput").ap()

    with tile.TileContext(nc) as tc:
        with (
            tc.tile_pool(name="const", bufs=1) as constp,
            tc.tile_pool(name="ktpool", bufs=IC) as ktp,
            tc.tile_pool(name="qtpool", bufs=IC) as qtp,
            tc.tile_pool(name="accp", bufs=2, space="PSUM") as accp,
            tc.tile_pool(name="tpp", bufs=2, space="PSUM") as tpp,
            tc.tile_pool(name="opp", bufs=4, space="PSUM") as opp,
            tc.tile_pool(name="dram", bufs=1, space="DRAM") as dramp,
        ):
            ident_sb = constp.tile([P, P], FP32R, name="ident_sb", tag="id")
            nc.sync.dma_start(out=ident_sb[:], in_=ident[:])
            b_sb = constp.tile([P, NO], FP32, name="b_sb", tag="b")
            nc.sync.dma_start(out=b_sb[:], in_=bb[:])

            KT = [ktp.tile([P, S], FP32R, name=f"kt{i}", tag="kt") for i in range(IC)]
            QT = [
                qtp.tile([P, NSUB * P], FP32R, name=f"qt{i}", tag="qt")
                for i in range(IC)
            ]
            # staging + exchange buffers for the pair K/V AllGathers
            ktst = dramp.tile([2, NI, 512], FP32R, name="ktst", tag="ktst")
            ktg = dramp.tile([2, 2 * NI, 512], FP32R, name="ktg", tag="ktg")
            vst = dramp.tile([2, 512, NI], FP32R, name="vst", tag="vst")
            v_dram = dramp.tile([S, NI], FP32R, name="v_dram", tag="vd")

            with tc.tile_pool(name="wpool", bufs=2 * DC) as wp:
                def load_w(src, label):
                    # halved DMAs: a 512KB tile on one HWDGE queue is ~19us
                    ts = []
                    for d in range(DC):
                        t = wp.tile([P, NI], FP32R, name=f"{label}{d}", tag="w")
                        for q in range(2):
                            nc.sync.dma_start(
                                out=t[:, 512 * q:512 * (q + 1)],
                                in_=src[P * d:P * (d + 1), 512 * q:512 * (q + 1)],
                            )
                        ts.append(t)
                    return ts

                with (
                    tc.tile_pool(name="xqp", bufs=DC) as xqp,
                    tc.tile_pool(name="xtp", bufs=DC) as xtp,
                    tc.tile_pool(name="stagep", bufs=4) as stp_,
                ):
                    # ---- Phase 0: Q^T for all 1024 local queries ----
                    wq = load_w(wq_d, "wq")

                    def load_xcols(src, col0, label):
                        ts = []
                        for d in range(DC):
                            t = xtp.tile(
                                [P, 512], FP32R, name=f"{label}{d}", tag="xt",
                            ) if label.startswith("xm") else xqp.tile(
                                [P, 512], FP32R, name=f"{label}{d}", tag="xq",
                            )
                            for q in range(2):
                                nc.sync.dma_start(
                                    out=t[:, 256 * q:256 * (q + 1)],
                                    in_=src[P * d:P * (d + 1),
                                            col0 + 256 * q:col0 + 256 * (q + 1)],
                                )
                            ts.append(t)
                        return ts

                    xt0 = load_xcols(xM, 0, "xm0_")  # prefetch first K/V chunk
                    wk = load_w(wk_d, "wk")

                    for qh in range(2):
                        xqs = load_xcols(xQ, 512 * qh, f"xq{qh}_")
                        for i in range(IC):
                            ps = accp.tile([P, 512], FP32, name="ps_qt", tag="acc")
                            for d in range(DC):
                                nc.tensor.matmul(
                                    ps[:], wq[d][:, P * i:P * (i + 1)], xqs[d][:],
                                    start=(d == 0), stop=(d == DC - 1),
                                )
                            nc.vector.tensor_copy(
                                QT[i][:, 512 * qh:512 * (qh + 1)], ps[:]
                            )

                    # ---- Phase 1: my half of K^T and V + pair AllGathers ----
                    wv = load_w(wv_d, "wv")  # reuses wq's slots after Q^T

                    for myc in range(2):
                        xts = xt0 if myc == 0 else load_xcols(xM, 512, "xm1_")
                        for i in range(IC):
                            ps = accp.tile([P, 512], FP32, name="ps_kt", tag="acc")
                            for d in range(DC):
                                nc.tensor.matmul(
                                    ps[:], wk[d][:, P * i:P * (i + 1)], xts[d][:],
                                    start=(d == 0), stop=(d == DC - 1),
                                )
                            st = stp_.tile([P, 512], FP32R, name="kst", tag="st")
                            nc.vector.tensor_copy(st[:], ps[:])
                            nc.sync.dma_start(
                                out=ktst[myc, P * i:P * (i + 1), :], in_=st[:]
                            )
                        for vs in range(4):
                            for ih in range(2):
                                ps = accp.tile([P, 512], FP32, name="ps_v", tag="acc")
                                for d in range(DC):
                                    nc.tensor.matmul(
                                        ps[:],
                                        xts[d][:, P * vs:P * (vs + 1)],
                                        wv[d][:, 512 * ih:512 * (ih + 1)],
                                        start=(d == 0), stop=(d == DC - 1),
                                    )
                                st = stp_.tile([P, 512], FP32R, name="vst_t", tag="st")
                                nc.vector.tensor_copy(st[:], ps[:])
                                nc.sync.dma_start(
                                    out=vst[myc, P * vs:P * (vs + 1),
                                            512 * ih:512 * (ih + 1)],
                                    in_=st[:],
                                )
                        # exchange this 512-row chunk with the pair partner
                        nc.gpsimd.collective_compute(
                            "AllGather", ALU.bypass, replica_groups=PAIRS,
                            ins=[ktst[myc].opt()], outs=[ktg[myc].opt()],
                        )
                        nc.gpsimd.collective_compute(
                            "AllGather", ALU.bypass, replica_groups=PAIRS,
                            ins=[vst[myc].opt()],
                            outs=[v_dram[1024 * myc:1024 * (myc + 1), :].opt()],
                        )
                        for r in range(2):
                            rc = 2 * myc + r
                            for i in range(IC):
                                nc.sync.dma_start(
                                    out=KT[i][:, 512 * rc:512 * (rc + 1)],
                                    in_=ktg[myc, NI * r + P * i:NI * r + P * (i + 1), :],
                                )

            # ---- attention, 4 pair-groups of 2 subtiles ----
            with tc.tile_pool(name="wopool", bufs=DC) as wop:
                wo = []
                for d in range(DC):
                    t = wop.tile([P, NI], FP32R, name=f"wo{d}", tag="wo")
                    for q in range(2):
                        nc.sync.dma_start(
                            out=t[:, 512 * q:512 * (q + 1)],
                            in_=wo_d[P * d:P * (d + 1), 512 * q:512 * (q + 1)],
                        )
                    wo.append(t)
                with (
                    tc.tile_pool(name="ppool", bufs=3) as ppool,
                    tc.tile_pool(name="ptpool", bufs=4) as ptpool,
                    tc.tile_pool(name="otpool", bufs=10) as otpool,
                    tc.tile_pool(name="vrd", bufs=3) as vrdp,
                    tc.tile_pool(name="mskp", bufs=2) as mskp,
                    tc.tile_pool(name="ypool", bufs=2) as ypool,
                    tc.tile_pool(name="stp", bufs=4) as stp,
                ):
                    for g in range(4):
                        L = g + 1
                        k0, k1 = 2 * g, 2 * g + 1
                        Ps = {}
                        for k in (k0, k1):
                            p_t = ppool.tile([P, 4 * 512], FP32R, name=f"p{k}", tag="p")
                            sums = stp.tile([P, 4], FP32, name=f"sums{k}", tag="sums")
                            for kc in range(L):
                                ps = accp.tile([P, 512], FP32, name="ps_sim", tag="acc")
                                for i in range(IC):
                                    nc.tensor.matmul(
                                        ps[:],
                                        QT[i][:, P * k:P * (k + 1)],
                                        KT[i][:, 512 * kc:512 * (kc + 1)],
                                        start=(i == 0), stop=(i == IC - 1),
                                    )
                                if kc == L - 1:
                                    m_t = mskp.tile([P, 512], FP32, name="m_t", tag="m")
                                    nc.sync.dma_start(out=m_t[:], in_=masks[k])
                                    nc.vector.tensor_tensor(
                                        out=ps[:], in0=ps[:], in1=m_t[:], op=ALU.add
                                    )
                                nc.scalar.activation(
                                    p_t[:, 512 * kc:512 * (kc + 1)], ps[:], AF.Exp,
                                    scale=SCALE, accum_out=sums[:, kc:kc + 1],
                                )
                            ssum = stp.tile([P, 1], FP32, name=f"ssum{k}", tag="ss")
                            nc.vector.tensor_reduce(
                                ssum[:], sums[:, :L], axis=mybir.AxisListType.X,
                                op=ALU.add,
                            )
                            rsum = stp.tile([P, 1], FP32, name=f"rsum{k}", tag="rs")
                            nc.vector.reciprocal(rsum[:], ssum[:])
                            nc.vector.tensor_scalar_mul(
                                p_t[:, :512 * L], p_t[:, :512 * L], rsum[:]
                            )
                            Ps[k] = p_t

                        ops = [
                            opp.tile([P, 512], FP32, name=f"op{g}_{j}", tag="op")
                            for j in range(4)
                        ]
                        nt = 4 * L
                        for t in range(nt):
                            tp_ps = tpp.tile([P, 256], FP32R, name="tp", tag="tp")
                            nc.tensor.transpose(
                                tp_ps[:, 0:P], Ps[k0][:, P * t:P * (t + 1)], ident_sb[:]
                            )
                            nc.tensor.transpose(
                                tp_ps[:, P:256], Ps[k1][:, P * t:P * (t + 1)], ident_sb[:]
                            )
                            pt_t = ptpool.tile([P, 256], FP32R, name="pt", tag="pt")
                            nc.vector.tensor_copy(pt_t[:], tp_ps[:])
                            v_t = vrdp.tile([P, NI], FP32R, name="v_t", tag="v")
                            nc.sync.dma_start(
                                out=v_t[:], in_=v_dram[P * t:P * (t + 1), :]
                            )
                            for m in range(IC):
                                # one accumulation group per PSUM bank: start
                                # only on the bank's first matmul (whole-bank
                                # pending-zero makes the sibling column-half's
                                # first write an overwrite), stop on its last
                                nc.tensor.matmul(
                                    ops[m // 2][:, 256 * (m % 2):256 * (m % 2) + 256],
                                    v_t[:, P * m:P * (m + 1)],
                                    pt_t[:],
                                    start=(t == 0 and m % 2 == 0),
                                    stop=(t == nt - 1 and m % 2 == 1),
                                )

                        oT = []
                        for m in range(IC):
                            ot = otpool.tile([P, 256], FP32R, name=f"ot{g}_{m}", tag="ot")
                            nc.vector.tensor_copy(
                                ot[:], ops[m // 2][:, 256 * (m % 2):256 * (m % 2) + 256]
                            )
                            oT.append(ot)

                        # ---- output projection for this group's 2 subtiles ----
                        for col, k in enumerate((k0, k1)):
                            y_sb = ypool.tile([P, NO], FP32, name="y_sb", tag="y")
                            for oh in range(2):
                                ps = accp.tile([P, 512], FP32, name="ps_y", tag="acc")
                                for i in range(IC):
                                    nc.tensor.matmul(
                                        ps[:],
                                        oT[i][:, P * col:P * (col + 1)],
                                        wo[i][:, 512 * oh:512 * (oh + 1)],
                                        start=(i == 0), stop=(i == IC - 1),
                                    )
                                nc.vector.tensor_tensor(
                                    out=y_sb[:, 512 * oh:512 * (oh + 1)], in0=ps[:],
                                    in1=b_sb[:, 512 * oh:512 * (oh + 1)], op=ALU.add,
                                )
                            nc.sync.dma_start(out=y[P * k:P * (k + 1), :], in_=y_sb[:])

    nc.compile()
    return nc


def _prep_inputs(x, w_qkv, w_out, b_out):
    x = np.asarray(x, dtype=np.float32)
    w_qkv = np.asarray(w_qkv, dtype=np.float32)
    w_out = np.asarray(w_out, dtype=np.float32)
    b_out = np.asarray(b_out, dtype=np.float32)

    wq = np.ascontiguousarray(w_qkv[:, 0 * NI:1 * NI])
    wk = np.ascontiguousarray(w_qkv[:, 1 * NI:2 * NI])
    wv = np.ascontiguousarray(w_qkv[:, 2 * NI:3 * NI])
    b_bcast = np.ascontiguousarray(np.broadcast_to(b_out[None, :], (P, NO)))
    ident = np.eye(P, dtype=np.float32)

    xTs = [np.ascontiguousarray(x[b].T) for b in range(B)]

    in_maps = []
    for c in range(NCORES):
        b, h = c // 2, c % 2
        subs = [2 * k + h for k in range(NSUB)]
        xQ = np.concatenate(
            [xTs[b][:, P * s:P * (s + 1)] for s in subs], axis=1
        )
        # my K/V production chunks: global 512-row chunks {h, 2+h}
        xM = np.concatenate(
            [xTs[b][:, 512 * h:512 * (h + 1)],
             xTs[b][:, 512 * (2 + h):512 * (3 + h)]], axis=1
        )
        m = np.empty((NSUB, P, 512), dtype=np.float32)
        cpos = np.arange(512)[None, :]
        prow = np.arange(P)[:, None]
        for k in range(NSUB):
            off = P * subs[k] - 512 * (CC[k] - 1)
            m[k] = np.where(cpos <= off + prow, 0.0, NEG)
        in_maps.append({
            "xM": np.ascontiguousarray(xM), "xQ": np.ascontiguousarray(xQ),
            "wk": wk, "wv": wv, "wq": wq, "wo": w_out,
            "masks": m, "bb": b_bcast, "ident": ident,
        })
    return in_maps


def _run(x, w_qkv, w_out, b_out, trace=False, **kw):
    if "nc" not in _CACHED:
        _CACHED["nc"] = _build()
    nc = _CACHED["nc"]
    in_maps = _prep_inputs(x, w_qkv, w_out, b_out)
    res = run_bass_kernel_spmd(nc, in_maps, list(range(NCORES)), trace=trace, **kw)
    out = np.empty((B, S, NO), dtype=np.float32)
    for c in range(NCORES):
        b, h = c // 2, c % 2
        yc = res.results[c]["y"]
        for k in range(NSUB):
            s = 2 * k + h
            out[b, P * s:P * (s + 1), :] = yc[P * k:P * (k + 1), :]
    return out, res


def kernel(x, w_qkv, w_out, b_out):
    out, _ = _run(x, w_qkv, w_out, b_out, trace=False)
    return out


# revision 10
# speedup vs baseline: 1.4153x; 1.4153x over previous
"""Causal single-head attention block for Trainium2, SPMD across 8 NeuronCores.

Problem (hardcoded):
    x:     [4, 2048, 1024] f32
    w_qkv: [1024, 3072]    f32   (q | k | v column blocks)
    w_out: [1024, 1024]    f32
    b_out: [1024]          f32
    y = softmax(causal(q @ k.T / 32)) @ v @ w_out + b_out     -> [4, 2048, 1024]

Sharding: 2 cores per batch element. Within a batch, the 16 query subtiles of
128 rows are dealt round-robin to the core pair (core parity h gets subtiles
s = 2k + h, k = 0..7) so both cores see the identical causal work profile
(key-chunk counts [1,1,2,2,3,3,4,4]) and a single SPMD program serves all 8
cores; per-core behavior differs only through input data.

K/V production is split across the pair: each core projects only its parity's
two 512-row chunks (global chunks h and 2+h) and the pair exchanges them with
two AllGathers (one per 1024-row half, so the first half arrives while the
second is still being produced). K^T lives in SBUF; V is streamed from DRAM
during attention. All matmuls run in float32r (TF32-like PE mode, 4x the
fp32 matmul rate).
"""

import numpy as np

import concourse.mybir as mybir
import concourse.tile as tile
from concourse import bacc
from concourse.bass_utils import run_bass_kernel_spmd

FP32 = mybir.dt.float32
FP32R = mybir.dt.float32r
AF = mybir.ActivationFunctionType
ALU = mybir.AluOpType

B, S, D, NI, NO = 4, 2048, 1024, 1024, 1024
NCORES = 8
P = 128
DC = D // P    # 8 contraction chunks for the projections
IC = NI // P   # 8 inner-dim chunks
RC = S // 512  # 4 key/row production chunks
NSUB = 8       # local 128-row query subtiles per core
CC = [k // 2 + 1 for k in range(NSUB)]  # 512-key chunks per local subtile
SCALE = float(NI) ** -0.5
NEG = -1.0e9
PAIRS = [[2 * b, 2 * b + 1] for b in range(B)]

_CACHED = {}


def _build():
    nc = bacc.Bacc(None, target_bir_lowering=False, debug=False, num_devices=NCORES)

    xT = nc.dram_tensor("xT", [D, S], FP32R, kind="ExternalInput").ap()
    xQ = nc.dram_tensor("xQ", [D, NSUB * P], FP32R, kind="ExternalInput").ap()
    wk_d = nc.dram_tensor("wk", [D, NI], FP32R, kind="ExternalInput").ap()
    wv_d = nc.dram_tensor("wv", [D, NI], FP32R, kind="ExternalInput").ap()
    wq_d = nc.dram_tensor("wq", [D, NI], FP32R, kind="ExternalInput").ap()
    wo_d = nc.dram_tensor("wo", [NI, NO], FP32R, kind="ExternalInput").ap()
    masks = nc.dram_tensor("masks", [NSUB, P, 512], FP32, kind="ExternalInput").ap()
    bb = nc.dram_tensor("bb", [P, NO], FP32, kind="ExternalInput").ap()
    ident = nc.dram_tensor("ident", [P, P], FP32R, kind="ExternalInput").ap()
    y = nc.dram_tensor("y", [NSUB * P, NO], FP32, kind="ExternalOutput").ap()

    with tile.TileContext(nc) as tc:
        with (
            tc.tile_pool(name="const", bufs=1) as constp,
            tc.tile_pool(name="ktpool", bufs=IC) as ktp,
            tc.tile_pool(name="qtpool", bufs=IC) as qtp,
            tc.tile_pool(name="accp", bufs=2, space="PSUM") as accp,
            tc.tile_pool(name="tpp", bufs=2, space="PSUM") as tpp,
            tc.tile_pool(name="opp", bufs=4, space="PSUM") as opp,
            tc.tile_pool(name="dram", bufs=1, space="DRAM") as dramp,
        ):
            ident_sb = constp.tile([P, P], FP32R, name="ident_sb", tag="id")
            nc.sync.dma_start(out=ident_sb[:], in_=ident[:])
            b_sb = constp.tile([P, NO], FP32, name="b_sb", tag="b")
            nc.sync.dma_start(out=b_sb[:], in_=bb[:])

            KT = [ktp.tile([P, S], FP32R, name=f"kt{i}", tag="kt") for i in range(IC)]
            QT = [
                qtp.tile([P, NSUB * P], FP32R, name=f"qt{i}", tag="qt")
                for i in range(IC)
            ]
            v_dram = dramp.tile([S, NI], FP32R, name="v_dram", tag="vd")

            with tc.tile_pool(name="wpool", bufs=2 * DC) as wp:
                def load_w(src, label, nsplit):
                    ts = []
                    for d in range(DC):
                        t = wp.tile([P, NI], FP32R, name=f"{label}{d}", tag="w")
                        w_ = NI // nsplit
                        for q in range(nsplit):
                            nc.sync.dma_start(
                                out=t[:, w_ * q:w_ * (q + 1)],
                                in_=src[P * d:P * (d + 1), w_ * q:w_ * (q + 1)],
                            )
                        ts.append(t)
                    return ts

                with tc.tile_pool(name="xtp", bufs=12) as xtp:
                    def load_xt(rc):
                        ts = []
                        for d in range(DC):
                            t = xtp.tile([P, 512], FP32R, name=f"x{rc}_{d}", tag="xt")
                            nc.sync.dma_start(
                                out=t[:],
                                in_=xT[P * d:P * (d + 1), 512 * rc:512 * (rc + 1)],
                            )
                            ts.append(t)
                        return ts

                    # ---- Phase 0: Q^T for all 1024 local queries ----
                    with tc.tile_pool(name="xqp", bufs=DC) as xqp:
                        wq = load_w(wq_d, "wq", 4)
                        for qh in range(2):
                            xqs = []
                            for d in range(DC):
                                t = xqp.tile([P, 512], FP32R, name=f"xq{qh}_{d}",
                                             tag="xq")
                                for q in range(2):
                                    nc.sync.dma_start(
                                        out=t[:, 256 * q:256 * (q + 1)],
                                        in_=xQ[P * d:P * (d + 1),
                                               512 * qh + 256 * q:
                                               512 * qh + 256 * (q + 1)],
                                    )
                                xqs.append(t)
                            if qh == 0:
                                xt0 = load_xt(0)      # prefetch K/V chunk 0
                                wk = load_w(wk_d, "wk", 2)
                            for i in range(IC):
                                ps = accp.tile([P, 512], FP32, name="ps_qt", tag="acc")
                                for d in range(DC):
                                    nc.tensor.matmul(
                                        ps[:], wq[d][:, P * i:P * (i + 1)], xqs[d][:],
                                        start=(d == 0), stop=(d == DC - 1),
                                    )
                                nc.vector.tensor_copy(
                                    QT[i][:, 512 * qh:512 * (qh + 1)], ps[:]
                                )

                    # ---- Phase 1: K^T (SBUF-resident) and V (DRAM) ----
                    with tc.tile_pool(name="vst", bufs=2) as vstp:
                        wv = load_w(wv_d, "wv", 2)  # reuses wq's slots after Q^T
                        for rc in range(RC):
                            xts = xt0 if rc == 0 else load_xt(rc)
                            for i in range(IC):
                                ps = accp.tile([P, 512], FP32, name="ps_kt", tag="acc")
                                for d in range(DC):
                                    nc.tensor.matmul(
                                        ps[:], wk[d][:, P * i:P * (i + 1)], xts[d][:],
                                        start=(d == 0), stop=(d == DC - 1),
                                    )
                                nc.vector.tensor_copy(
                                    KT[i][:, 512 * rc:512 * (rc + 1)], ps[:]
                                )
                            for vs in range(4):
                                vt = vstp.tile([P, NI], FP32R, name="vstage", tag="vst")
                                for ih in range(2):
                                    ps = accp.tile([P, 512], FP32, name="ps_v",
                                                   tag="acc")
                                    for d in range(DC):
                                        nc.tensor.matmul(
                                            ps[:],
                                            xts[d][:, P * vs:P * (vs + 1)],
                                            wv[d][:, 512 * ih:512 * (ih + 1)],
                                            start=(d == 0), stop=(d == DC - 1),
                                        )
                                    nc.vector.tensor_copy(
                                        vt[:, 512 * ih:512 * (ih + 1)], ps[:]
                                    )
                                row = 512 * rc + P * vs
                                nc.sync.dma_start(out=v_dram[row:row + P, :], in_=vt[:])

            # ---- attention, 4 pair-groups of 2 subtiles ----
            with tc.tile_pool(name="wopool", bufs=DC) as wop:
                wo = []
                for d in range(DC):
                    t = wop.tile([P, NI], FP32R, name=f"wo{d}", tag="wo")
                    for q in range(2):
                        nc.sync.dma_start(
                            out=t[:, 512 * q:512 * (q + 1)],
                            in_=wo_d[P * d:P * (d + 1), 512 * q:512 * (q + 1)],
                        )
                    wo.append(t)
                with (
                    tc.tile_pool(name="ppool", bufs=3) as ppool,
                    tc.tile_pool(name="ptpool", bufs=4) as ptpool,
                    tc.tile_pool(name="otpool", bufs=10) as otpool,
                    tc.tile_pool(name="vrd", bufs=3) as vrdp,
                    tc.tile_pool(name="mskp", bufs=2) as mskp,
                    tc.tile_pool(name="ypool", bufs=2) as ypool,
                    tc.tile_pool(name="stp", bufs=4) as stp,
                ):
                    for g in range(4):
                        L = g + 1
                        k0, k1 = 2 * g, 2 * g + 1
                        Ps = {}
                        for k in (k0, k1):
                            p_t = ppool.tile([P, 4 * 512], FP32R, name=f"p{k}", tag="p")
                            sums = stp.tile([P, 4], FP32, name=f"sums{k}", tag="sums")
                            for kc in range(L):
                                ps = accp.tile([P, 512], FP32, name="ps_sim", tag="acc")
                                for i in range(IC):
                                    nc.tensor.matmul(
                                        ps[:],
                                        QT[i][:, P * k:P * (k + 1)],
                                        KT[i][:, 512 * kc:512 * (kc + 1)],
                                        start=(i == 0), stop=(i == IC - 1),
                                    )
                                if kc == L - 1:
                                    m_t = mskp.tile([P, 512], FP32, name="m_t", tag="m")
                                    nc.sync.dma_start(out=m_t[:], in_=masks[k])
                                    nc.vector.tensor_tensor(
                                        out=ps[:], in0=ps[:], in1=m_t[:], op=ALU.add
                                    )
                                nc.scalar.activation(
                                    p_t[:, 512 * kc:512 * (kc + 1)], ps[:], AF.Exp,
                                    scale=SCALE, accum_out=sums[:, kc:kc + 1],
                                )
                            ssum = stp.tile([P, 1], FP32, name=f"ssum{k}", tag="ss")
                            nc.vector.tensor_reduce(
                                ssum[:], sums[:, :L], axis=mybir.AxisListType.X,
                                op=ALU.add,
                            )
                            rsum = stp.tile([P, 1], FP32, name=f"rsum{k}", tag="rs")
                            nc.vector.reciprocal(rsum[:], ssum[:])
                            nc.vector.tensor_scalar_mul(
                                p_t[:, :512 * L], p_t[:, :512 * L], rsum[:]
                            )
                            Ps[k] = p_t

                        ops = [
                            opp.tile([P, 512], FP32, name=f"op{g}_{j}", tag="op")
                            for j in range(4)
                        ]
                        nt = 4 * L
                        for t in range(nt):
                            tp_ps = tpp.tile([P, 256], FP32R, name="tp", tag="tp")
                            nc.tensor.transpose(
                                tp_ps[:, 0:P], Ps[k0][:, P * t:P * (t + 1)], ident_sb[:]
                            )
                            nc.tensor.transpose(
                                tp_ps[:, P:256], Ps[k1][:, P * t:P * (t + 1)], ident_sb[:]
                            )
                            pt_t = ptpool.tile([P, 256], FP32R, name="pt", tag="pt")
                            nc.vector.tensor_copy(pt_t[:], tp_ps[:])
                            v_t = vrdp.tile([P, NI], FP32R, name="v_t", tag="v")
                            nc.sync.dma_start(
                                out=v_t[:], in_=v_dram[P * t:P * (t + 1), :]
                            )
                            for m in range(IC):
                                # one accumulation group per PSUM bank: start
                                # only on the bank's first matmul (whole-bank
                                # pending-zero makes the sibling column-half's
                                # first write an overwrite), stop on its last
                                nc.tensor.matmul(
                                    ops[m // 2][:, 256 * (m % 2):256 * (m % 2) + 256],
                                    v_t[:, P * m:P * (m + 1)],
                                    pt_t[:],
                                    start=(t == 0 and m % 2 == 0),
                                    stop=(t == nt - 1 and m % 2 == 1),
                                )

                        oT = []
                        for m in range(IC):
                            ot = otpool.tile([P, 256], FP32R, name=f"ot{g}_{m}", tag="ot")
                            nc.vector.tensor_copy(
                                ot[:], ops[m // 2][:, 256 * (m % 2):256 * (m % 2) + 256]
                            )
                            oT.append(ot)

                        # ---- output projection for this group's 2 subtiles ----
                        for col, k in enumerate((k0, k1)):
                            y_sb = ypool.tile([P, NO], FP32, name="y_sb", tag="y")
                            for oh in range(2):
                                ps = accp.tile([P, 512], FP32, name="ps_y", tag="acc")
                                for i in range(IC):
                                    nc.tensor.matmul(
                                        ps[:],
                                        oT[i][:, P * col:P * (col + 1)],
                                        wo[i][:, 512 * oh:512 * (oh + 1)],
                                        start=(i == 0), stop=(i == IC - 1),
                                    )
                                nc.vector.tensor_tensor(
                                    out=y_sb[:, 512 * oh:512 * (oh + 1)], in0=ps[:],
                                    in1=b_sb[:, 512 * oh:512 * (oh + 1)], op=ALU.add,
                                )
                            nc.sync.dma_start(out=y[P * k:P * (k + 1), :], in_=y_sb[:])

    nc.compile()
    return nc


def _prep_inputs(x, w_qkv, w_out, b_out):
    x = np.asarray(x, dtype=np.float32)
    w_qkv = np.asarray(w_qkv, dtype=np.float32)
    w_out = np.asarray(w_out, dtype=np.float32)
    b_out = np.asarray(b_out, dtype=np.float32)

    wq = np.ascontiguousarray(w_qkv[:, 0 * NI:1 * NI])
    wk = np.ascontiguousarray(w_qkv[:, 1 * NI:2 * NI])
    wv = np.ascontiguousarray(w_qkv[:, 2 * NI:3 * NI])
    b_bcast = np.ascontiguousarray(np.broadcast_to(b_out[None, :], (P, NO)))
    ident = np.eye(P, dtype=np.float32)

    xTs = [np.ascontiguousarray(x[b].T) for b in range(B)]

    in_maps = []
    for c in range(NCORES):
        b, h = c // 2, c % 2
        subs = [2 * k + h for k in range(NSUB)]
        xQ = np.concatenate(
            [xTs[b][:, P * s:P * (s + 1)] for s in subs], axis=1
        )
        m = np.empty((NSUB, P, 512), dtype=np.float32)
        cpos = np.arange(512)[None, :]
        prow = np.arange(P)[:, None]
        for k in range(NSUB):
            off = P * subs[k] - 512 * (CC[k] - 1)
            m[k] = np.where(cpos <= off + prow, 0.0, NEG)
        in_maps.append({
            "xT": xTs[b], "xQ": np.ascontiguousarray(xQ),
            "wk": wk, "wv": wv, "wq": wq, "wo": w_out,
            "masks": m, "bb": b_bcast, "ident": ident,
        })
    return in_maps


def _run(x, w_qkv, w_out, b_out, trace=False, **kw):
    if "nc" not in _CACHED:
        _CACHED["nc"] = _build()
    nc = _CACHED["nc"]
    in_maps = _prep_inputs(x, w_qkv, w_out, b_out)
    res = run_bass_kernel_spmd(nc, in_maps, list(range(NCORES)), trace=trace, **kw)
    out = np.empty((B, S, NO), dtype=np.float32)
    for c in range(NCORES):
        b, h = c // 2, c % 2
        yc = res.results[c]["y"]
        for k in range(NSUB):
            s = 2 * k + h
            out[b, P * s:P * (s + 1), :] = yc[P * k:P * (k + 1), :]
    return out, res


def kernel(x, w_qkv, w_out, b_out):
    out, _ = _run(x, w_qkv, w_out, b_out, trace=False)
    return out


# revision 12
# speedup vs baseline: 1.5636x; 1.1048x over previous
"""Causal single-head attention block for Trainium2, SPMD across 8 NeuronCores.

Problem (hardcoded):
    x:     [4, 2048, 1024] f32
    w_qkv: [1024, 3072]    f32   (q | k | v column blocks)
    w_out: [1024, 1024]    f32
    b_out: [1024]          f32
    y = softmax(causal(q @ k.T / 32)) @ v @ w_out + b_out     -> [4, 2048, 1024]

Sharding: 2 cores per batch element. Within a batch, the 16 query subtiles of
128 rows are dealt round-robin to the core pair (core parity h gets subtiles
s = 2k + h, k = 0..7) so both cores see the identical causal work profile
(key-chunk counts [1,1,2,2,3,3,4,4]) and a single SPMD program serves all 8
cores; per-core behavior differs only through input data.

K/V production is split across the pair: each core projects only its parity's
two 512-row chunks (global chunks h and 2+h) and the pair exchanges them with
two AllGathers (one per 1024-row half, so the first half arrives while the
second is still being produced). K^T lives in SBUF; V is streamed from DRAM
during attention. All matmuls run in float32r (TF32-like PE mode, 4x the
fp32 matmul rate).
"""

import numpy as np

import concourse.mybir as mybir
import concourse.tile as tile
from concourse import bacc
from concourse.bass_utils import run_bass_kernel_spmd

FP32 = mybir.dt.float32
FP32R = mybir.dt.float32r
AF = mybir.ActivationFunctionType
ALU = mybir.AluOpType

B, S, D, NI, NO = 4, 2048, 1024, 1024, 1024
NCORES = 8
P = 128
DC = D // P    # 8 contraction chunks for the projections
IC = NI // P   # 8 inner-dim chunks
RC = S // 512  # 4 key/row production chunks
NSUB = 8       # local 128-row query subtiles per core
CC = [k // 2 + 1 for k in range(NSUB)]  # 512-key chunks per local subtile
SCALE = float(NI) ** -0.5
NEG = -1.0e9
PAIRS = [[2 * b, 2 * b + 1] for b in range(B)]

_CACHED = {}


def _build():
    nc = bacc.Bacc(None, target_bir_lowering=False, debug=False, num_devices=NCORES)

    xT = nc.dram_tensor("xT", [D, S], FP32R, kind="ExternalInput").ap()
    xQ = nc.dram_tensor("xQ", [D, NSUB * P], FP32R, kind="ExternalInput").ap()
    wk_d = nc.dram_tensor("wk", [D, NI], FP32R, kind="ExternalInput").ap()
    wv_d = nc.dram_tensor("wv", [D, NI], FP32R, kind="ExternalInput").ap()
    wq_d = nc.dram_tensor("wq", [D, NI], FP32R, kind="ExternalInput").ap()
    wo_d = nc.dram_tensor("wo", [NI, NO], FP32R, kind="ExternalInput").ap()
    masks = nc.dram_tensor("masks", [NSUB, P, 512], FP32, kind="ExternalInput").ap()
    bb = nc.dram_tensor("bb", [P, NO], FP32, kind="ExternalInput").ap()
    ident = nc.dram_tensor("ident", [P, P], FP32R, kind="ExternalInput").ap()
    y = nc.dram_tensor("y", [NSUB * P, NO], FP32, kind="ExternalOutput").ap()

    with tile.TileContext(nc) as tc:
        with (
            tc.tile_pool(name="const", bufs=1) as constp,
            tc.tile_pool(name="ktpool", bufs=IC) as ktp,
            tc.tile_pool(name="qtpool", bufs=IC) as qtp,
            tc.tile_pool(name="accp", bufs=2, space="PSUM") as accp,
            tc.tile_pool(name="tpp", bufs=2, space="PSUM") as tpp,
            tc.tile_pool(name="opp", bufs=4, space="PSUM") as opp,
            tc.tile_pool(name="dram", bufs=1, space="DRAM") as dramp,
        ):
            ident_sb = constp.tile([P, P], FP32R, name="ident_sb", tag="id")
            nc.sync.dma_start(out=ident_sb[:], in_=ident[:])
            b_sb = constp.tile([P, NO], FP32, name="b_sb", tag="b")
            nc.sync.dma_start(out=b_sb[:], in_=bb[:])

            KT = [ktp.tile([P, S], FP32R, name=f"kt{i}", tag="kt") for i in range(IC)]
            QT = [
                qtp.tile([P, NSUB * P], FP32R, name=f"qt{i}", tag="qt")
                for i in range(IC)
            ]
            v_dram = dramp.tile([S, NI], FP32R, name="v_dram", tag="vd")

            with tc.tile_pool(name="wpool", bufs=2 * DC) as wp:
                def load_w(src, label, nsplit):
                    ts = []
                    for d in range(DC):
                        t = wp.tile([P, NI], FP32R, name=f"{label}{d}", tag="w")
                        w_ = NI // nsplit
                        for q in range(nsplit):
                            nc.sync.dma_start(
                                out=t[:, w_ * q:w_ * (q + 1)],
                                in_=src[P * d:P * (d + 1), w_ * q:w_ * (q + 1)],
                            )
                        ts.append(t)
                    return ts

                with tc.tile_pool(name="xtp", bufs=12) as xtp:
                    def load_xt(rc):
                        ts = []
                        for d in range(DC):
                            t = xtp.tile([P, 512], FP32R, name=f"x{rc}_{d}", tag="xt")
                            nc.sync.dma_start(
                                out=t[:],
                                in_=xT[P * d:P * (d + 1), 512 * rc:512 * (rc + 1)],
                            )
                            ts.append(t)
                        return ts

                    # ---- Phase 0: Q^T for all 1024 local queries ----
                    with tc.tile_pool(name="xqp", bufs=DC) as xqp:
                        wq = []
                        for qh in range(2):
                            xqs = []
                            for d in range(DC):
                                if qh == 0:
                                    # interleave weight/activation loads per d so
                                    # the d-loop's inputs arrive in consumption order
                                    wt = wp.tile([P, NI], FP32R, name=f"wq{d}", tag="w")
                                    for q in range(4):
                                        nc.sync.dma_start(
                                            out=wt[:, 256 * q:256 * (q + 1)],
                                            in_=wq_d[P * d:P * (d + 1),
                                                     256 * q:256 * (q + 1)],
                                        )
                                    wq.append(wt)
                                t = xqp.tile([P, 512], FP32R, name=f"xq{qh}_{d}",
                                             tag="xq")
                                for q in range(2):
                                    nc.sync.dma_start(
                                        out=t[:, 256 * q:256 * (q + 1)],
                                        in_=xQ[P * d:P * (d + 1),
                                               512 * qh + 256 * q:
                                               512 * qh + 256 * (q + 1)],
                                    )
                                xqs.append(t)
                            if qh == 0:
                                xt0 = load_xt(0)      # prefetch K/V chunk 0
                                wk = load_w(wk_d, "wk", 2)
                            for i in range(IC):
                                ps = accp.tile([P, 512], FP32, name="ps_qt", tag="acc")
                                for d in range(DC):
                                    nc.tensor.matmul(
                                        ps[:], wq[d][:, P * i:P * (i + 1)], xqs[d][:],
                                        start=(d == 0), stop=(d == DC - 1),
                                    )
                                nc.vector.tensor_copy(
                                    QT[i][:, 512 * qh:512 * (qh + 1)], ps[:]
                                )

                    # ---- Phase 1: K^T (SBUF-resident) and V (DRAM) ----
                    with tc.tile_pool(name="vst", bufs=2) as vstp:
                        wv = load_w(wv_d, "wv", 2)  # reuses wq's slots after Q^T
                        for rc in range(RC):
                            xts = xt0 if rc == 0 else load_xt(rc)
                            for i in range(IC):
                                ps = accp.tile([P, 512], FP32, name="ps_kt", tag="acc")
                                for d in range(DC):
                                    nc.tensor.matmul(
                                        ps[:], wk[d][:, P * i:P * (i + 1)], xts[d][:],
                                        start=(d == 0), stop=(d == DC - 1),
                                    )
                                nc.vector.tensor_copy(
                                    KT[i][:, 512 * rc:512 * (rc + 1)], ps[:]
                                )
                            for vs in range(4):
                                vt = vstp.tile([P, NI], FP32R, name="vstage", tag="vst")
                                for ih in range(2):
                                    ps = accp.tile([P, 512], FP32, name="ps_v",
                                                   tag="acc")
                                    for d in range(DC):
                                        nc.tensor.matmul(
                                            ps[:],
                                            xts[d][:, P * vs:P * (vs + 1)],
                                            wv[d][:, 512 * ih:512 * (ih + 1)],
                                            start=(d == 0), stop=(d == DC - 1),
                                        )
                                    nc.vector.tensor_copy(
                                        vt[:, 512 * ih:512 * (ih + 1)], ps[:]
                                    )
                                row = 512 * rc + P * vs
                                for q in range(2):
                                    nc.sync.dma_start(
                                        out=v_dram[row:row + P, 512 * q:512 * (q + 1)],
                                        in_=vt[:, 512 * q:512 * (q + 1)],
                                    )

            # ---- attention, 4 pair-groups of 2 subtiles ----
            with tc.tile_pool(name="wopool", bufs=DC) as wop:
                wo = []
                for d in range(DC):
                    t = wop.tile([P, NI], FP32R, name=f"wo{d}", tag="wo")
                    for q in range(2):
                        nc.sync.dma_start(
                            out=t[:, 512 * q:512 * (q + 1)],
                            in_=wo_d[P * d:P * (d + 1), 512 * q:512 * (q + 1)],
                        )
                    wo.append(t)
                with (
                    tc.tile_pool(name="ppool", bufs=2) as ppool,
                    tc.tile_pool(name="ptpool", bufs=4) as ptpool,
                    tc.tile_pool(name="otpool", bufs=10) as otpool,
                    tc.tile_pool(name="vrd", bufs=6) as vrdp,
                    tc.tile_pool(name="mskp", bufs=2) as mskp,
                    tc.tile_pool(name="ypool", bufs=2) as ypool,
                    tc.tile_pool(name="stp", bufs=4) as stp,
                ):
                    for g in range(4):
                        L = g + 1
                        k0, k1 = 2 * g, 2 * g + 1
                        Ps = {}
                        for k in (k0, k1):
                            p_t = ppool.tile([P, 4 * 512], FP32R, name=f"p{k}", tag="p")
                            sums = stp.tile([P, 4], FP32, name=f"sums{k}", tag="sums")
                            for kc in range(L):
                                ps = accp.tile([P, 512], FP32, name="ps_sim", tag="acc")
                                for i in range(IC):
                                    nc.tensor.matmul(
                                        ps[:],
                                        QT[i][:, P * k:P * (k + 1)],
                                        KT[i][:, 512 * kc:512 * (kc + 1)],
                                        start=(i == 0), stop=(i == IC - 1),
                                    )
                                if kc == L - 1:
                                    m_t = mskp.tile([P, 512], FP32, name="m_t", tag="m")
                                    nc.sync.dma_start(out=m_t[:], in_=masks[k])
                                    nc.vector.tensor_tensor(
                                        out=ps[:], in0=ps[:], in1=m_t[:], op=ALU.add
                                    )
                                nc.scalar.activation(
                                    p_t[:, 512 * kc:512 * (kc + 1)], ps[:], AF.Exp,
                                    scale=SCALE, accum_out=sums[:, kc:kc + 1],
                                )
                            ssum = stp.tile([P, 1], FP32, name=f"ssum{k}", tag="ss")
                            nc.vector.tensor_reduce(
                                ssum[:], sums[:, :L], axis=mybir.AxisListType.X,
                                op=ALU.add,
                            )
                            rsum = stp.tile([P, 1], FP32, name=f"rsum{k}", tag="rs")
                            nc.vector.reciprocal(rsum[:], ssum[:])
                            nc.vector.tensor_scalar_mul(
                                p_t[:, :512 * L], p_t[:, :512 * L], rsum[:]
                            )
                            Ps[k] = p_t

                        ops = [
                            opp.tile([P, 512], FP32, name=f"op{g}_{j}", tag="op")
                            for j in range(4)
                        ]
                        nt = 4 * L
                        for t in range(nt):
                            tp_ps = tpp.tile([P, 256], FP32R, name="tp", tag="tp")
                            nc.tensor.transpose(
                                tp_ps[:, 0:P], Ps[k0][:, P * t:P * (t + 1)], ident_sb[:]
                            )
                            nc.tensor.transpose(
                                tp_ps[:, P:256], Ps[k1][:, P * t:P * (t + 1)], ident_sb[:]
                            )
                            pt_t = ptpool.tile([P, 256], FP32R, name="pt", tag="pt")
                            nc.vector.tensor_copy(pt_t[:], tp_ps[:])
                            v_t = vrdp.tile([P, NI], FP32R, name="v_t", tag="v")
                            for q in range(2):
                                nc.sync.dma_start(
                                    out=v_t[:, 512 * q:512 * (q + 1)],
                                    in_=v_dram[P * t:P * (t + 1),
                                               512 * q:512 * (q + 1)],
                                )
                            for m in range(IC):
                                # one accumulation group per PSUM bank: start
                                # only on the bank's first matmul (whole-bank
                                # pending-zero makes the sibling column-half's
                                # first write an overwrite), stop on its last
                                nc.tensor.matmul(
                                    ops[m // 2][:, 256 * (m % 2):256 * (m % 2) + 256],
                                    v_t[:, P * m:P * (m + 1)],
                                    pt_t[:],
                                    start=(t == 0 and m % 2 == 0),
                                    stop=(t == nt - 1 and m % 2 == 1),
                                )

                        oT = []
                        for m in range(IC):
                            ot = otpool.tile([P, 256], FP32R, name=f"ot{g}_{m}", tag="ot")
                            nc.vector.tensor_copy(
                                ot[:], ops[m // 2][:, 256 * (m % 2):256 * (m % 2) + 256]
                            )
                            oT.append(ot)

                        # ---- output projection for this group's 2 subtiles ----
                        for col, k in enumerate((k0, k1)):
                            y_sb = ypool.tile([P, NO], FP32, name="y_sb", tag="y")
                            for oh in range(2):
                                ps = accp.tile([P, 512], FP32, name="ps_y", tag="acc")
                                for i in range(IC):
                                    nc.tensor.matmul(
                                        ps[:],
                                        oT[i][:, P * col:P * (col + 1)],
                                        wo[i][:, 512 * oh:512 * (oh + 1)],
                                        start=(i == 0), stop=(i == IC - 1),
                                    )
                                nc.vector.tensor_tensor(
                                    out=y_sb[:, 512 * oh:512 * (oh + 1)], in0=ps[:],
                                    in1=b_sb[:, 512 * oh:512 * (oh + 1)], op=ALU.add,
                                )
                            for q in range(2):
                                nc.sync.dma_start(
                                    out=y[P * k:P * (k + 1), 512 * q:512 * (q + 1)],
                                    in_=y_sb[:, 512 * q:512 * (q + 1)],
                                )

    nc.compile()
    return nc


def _prep_inputs(x, w_qkv, w_out, b_out):
    x = np.asarray(x, dtype=np.float32)
    w_qkv = np.asarray(w_qkv, dtype=np.float32)
    w_out = np.asarray(w_out, dtype=np.float32)
    b_out = np.asarray(b_out, dtype=np.float32)

    wq = np.ascontiguousarray(w_qkv[:, 0 * NI:1 * NI])
    wk = np.ascontiguousarray(w_qkv[:, 1 * NI:2 * NI])
    wv = np.ascontiguousarray(w_qkv[:, 2 * NI:3 * NI])
    b_bcast = np.ascontiguousarray(np.broadcast_to(b_out[None, :], (P, NO)))
    ident = np.eye(P, dtype=np.float32)

    xTs = [np.ascontiguousarray(x[b].T) for b in range(B)]

    in_maps = []
    for c in range(NCORES):
        b, h = c // 2, c % 2
        subs = [2 * k + h for k in range(NSUB)]
        xQ = np.concatenate(
            [xTs[b][:, P * s:P * (s + 1)] for s in subs], axis=1
        )
        m = np.empty((NSUB, P, 512), dtype=np.float32)
        cpos = np.arange(512)[None, :]
        prow = np.arange(P)[:, None]
        for k in range(NSUB):
            off = P * subs[k] - 512 * (CC[k] - 1)
            m[k] = np.where(cpos <= off + prow, 0.0, NEG)
        in_maps.append({
            "xT": xTs[b], "xQ": np.ascontiguousarray(xQ),
            "wk": wk, "wv": wv, "wq": wq, "wo": w_out,
            "masks": m, "bb": b_bcast, "ident": ident,
        })
    return in_maps


def _run(x, w_qkv, w_out, b_out, trace=False, **kw):
    if "nc" not in _CACHED:
        _CACHED["nc"] = _build()
    nc = _CACHED["nc"]
    in_maps = _prep_inputs(x, w_qkv, w_out, b_out)
    res = run_bass_kernel_spmd(nc, in_maps, list(range(NCORES)), trace=trace, **kw)
    out = np.empty((B, S, NO), dtype=np.float32)
    for c in range(NCORES):
        b, h = c // 2, c % 2
        yc = res.results[c]["y"]
        for k in range(NSUB):
            s = 2 * k + h
            out[b, P * s:P * (s + 1), :] = yc[P * k:P * (k + 1), :]
    return out, res


def kernel(x, w_qkv, w_out, b_out):
    out, _ = _run(x, w_qkv, w_out, b_out, trace=False)
    return out
